# revision 24
# baseline (speedup 1.0000x reference)
"""Self-contained Trainium2 (Bass) kernel for the 3-layer GCN
nn_FeaturePropagationModule problem: 100K nodes, 1.6M edges,
dims 512->64->128->40, log_softmax output, 8 NeuronCores.

Strategy (sharding_hint: shard nodes + edges by destination, replicate
weights): nodes are permuted into 8 shards x 98 blocks x 128 dsts
(degree-balanced); per layer each core computes y_own = (dis*h)@W for
its shard, AllGathers the bf16 y table (ONE collective per layer,
rank-major [PADN, TW] output), then aggregates its own dst blocks via
dma_gather of 256B source rows + one-hot S matmuls (scatter-add in
PSUM).

v4 design notes (from measured breakdowns):
- dma_gather is descriptor-LATENCY bound (~40ns/desc/engine with 4
  SWDGE queue rings in parallel; bytes nearly free <=512B; single
  queue is 3.8x slower). So: keep round-robin across the 4 queues,
  one ring tile per call, <=1024 idx per call (2048 desyncs the HW).
- AllGather floors dominated the baseline (12 AGs ~= 1.4ms/iter).
  v4 uses one table per layer; gather calls address the 4 rank-pair
  quarter-tables (25088 rows < 32768 so int16 indices still work);
  bucket(src) = owning core pair instead of a within-block class.
- Self-loops are never gathered: each block's PSUM accumulation
  starts with an identity matmul that adds the block's own table
  rows (ybuf), saving ~6% of gather descriptors.
- Epilogue fuses relu + symmetric-norm scaling (relu(s*x) = s*relu(x)
  for s>0); biases are zero per the problem spec (nonzero-bias inputs
  fall back to a numpy path). Final log_softmax on chip; host
  un-permutes rows.
"""
import heapq
import numpy as np

import concourse.bacc as bacc
import concourse.mybir as mybir
from concourse.bass_utils import run_bass_kernel_spmd
from concourse.masks import make_identity
from concourse.tile import TileContext

FP = mybir.dt.float32
BF = mybir.dt.bfloat16
I16 = mybir.dt.int16
TW = 128  # gather-table width (bf16 -> 256B rows)
NCORES = 8
N_NODES = 100000
NB_BLOCKS = 98
GRP = 7  # dst blocks per merged gather-call group


# ---------------- host-side preprocessing ----------------


def preprocess(edge_index: np.ndarray, N: int, NB: int, include_self=False):
    SHARD = NB * 128
    PADN = NCORES * SHARD
    QSIZE = PADN // 4  # rank-pair quarter table rows
    assert QSIZE < 32768 and N <= PADN
    src = edge_index[0].astype(np.int64)
    dst = edge_index[1].astype(np.int64)

    deg = np.bincount(dst, minlength=N).astype(np.float64) + 1.0
    dis = (1.0 / np.sqrt(deg)).astype(np.float32)
    if include_self:
        loop = np.arange(N, dtype=np.int64)
        src = np.concatenate([src, loop])
        dst = np.concatenate([dst, loop])

    # degree-balanced node -> (core, block) bin packing
    w = deg
    order = np.argsort(-w, kind="stable")
    nbins = NCORES * NB
    heap = [(0.0, b) for b in range(nbins)]
    heapq.heapify(heap)
    bin_nodes: list[list[int]] = [[] for _ in range(nbins)]
    bin_of_node = np.full(N, -1, dtype=np.int64)
    for n_ in order:
        while True:
            s_, b = heapq.heappop(heap)
            if len(bin_nodes[b]) < 128:
                break
        bin_nodes[b].append(int(n_))
        bin_of_node[n_] = b
        if len(bin_nodes[b]) < 128:
            heapq.heappush(heap, (s_ + w[n_], b))

    # Refine: rebin nodes to balance, for every (block, src-quarter) cell,
    # the in-edge count across cores (kbq is a max over cores, and cells
    # sit at ~510 edges vs the 512 four-chunk boundary, so variance is
    # what pushes cells to 5 chunks). In-edge quarter profiles depend on
    # placement, so iterate. Greedy: heaviest in-degree first, to the
    # open bin with the lowest resulting max-quarter count.
    for _it in range(2):
        qtr_of_node = (bin_of_node // NB) // 2
        prof = np.zeros((N, 4), dtype=np.float64)
        np.add.at(prof, (dst, qtr_of_node[src]), 1.0)
        order2 = np.argsort(-prof.sum(axis=1), kind="stable")
        bin_cell = np.zeros((nbins, 4), dtype=np.float64)
        bin_fill = np.zeros(nbins, dtype=np.int64)
        new_bin = np.full(N, -1, dtype=np.int64)
        bin_nodes = [[] for _ in range(nbins)]
        FULLPEN = 1e12
        for n_ in order2:
            sc = np.max(bin_cell + prof[n_], axis=1) \
                + 1e-4 * bin_fill + FULLPEN * (bin_fill >= 128)
            b = int(np.argmin(sc))
            new_bin[n_] = b
            bin_nodes[b].append(int(n_))
            bin_cell[b] += prof[n_]
            bin_fill[b] += 1
        bin_of_node = new_bin

    # Align block indices across cores: block labels are arbitrary per
    # core, and kbq is a max over cores at the same index — sort each
    # core's bins by their in-edge quarter profile so heavy cells
    # coincide and the max adds no padding.
    qtr_of_node = (bin_of_node // NB) // 2
    cellcnt = np.zeros((nbins, 4), dtype=np.int64)
    np.add.at(cellcnt, (bin_of_node[dst], qtr_of_node[src]), 1)
    blk_rank = np.zeros(nbins, dtype=np.int64)
    for c in range(NCORES):
        bins_c = np.arange(c * NB, (c + 1) * NB)
        keyv = np.ceil(cellcnt[bins_c] / 128.0)
        key = [tuple(-keyv[i]) + tuple(-cellcnt[bins_c][i])
               for i in range(NB)]
        order_c = sorted(range(NB), key=lambda i: key[i])
        for rank, i in enumerate(order_c):
            blk_rank[bins_c[i]] = rank

    perm_of_node = np.full(N, -1, dtype=np.int64)
    node_of_perm = np.full(PADN, -1, dtype=np.int64)
    for b in range(nbins):
        core = b // NB
        blk = int(blk_rank[b])
        base = core * SHARD + blk * 128
        for i, n_ in enumerate(bin_nodes[b]):
            perm_of_node[n_] = base + i
            node_of_perm[base + i] = n_
    assert (perm_of_node >= 0).all()

    # real edges only; self-loops are added on-chip
    psrc = perm_of_node[src]
    pdst = perm_of_node[dst]

    core_of = pdst // SHARD
    blk_of = (pdst % SHARD) // 128
    dcol_of = pdst % 128
    bucket_of = psrc // QSIZE  # src rank-pair

    counts = np.zeros((NCORES, NB, 4), dtype=np.int64)
    np.add.at(counts, (core_of, blk_of, bucket_of), 1)
    kbq = np.ceil(counts / 128).astype(np.int64).max(axis=0)  # [NB, 4]
    NCH = int(kbq.sum())

    # chunk-index maps.
    # dcol order  (block-major): for b, for q, for j in kbq[b,q]
    # idx order (group/bucket):  for g, for q, for b in g, for j
    kb = kbq.sum(axis=1)  # chunks per block
    dstart = np.zeros(NB + 1, dtype=np.int64)
    dstart[1:] = np.cumsum(kb)
    NGRP = NB // GRP
    assert NB % GRP == 0
    kgq = np.zeros((NGRP, 4), dtype=np.int64)
    for g in range(NGRP):
        for q in range(4):
            kgq[g, q] = kbq[g * GRP:(g + 1) * GRP, q].sum()
    chg = kgq.sum(axis=1)  # chunks per group
    gstart = np.zeros(NGRP + 1, dtype=np.int64)
    gstart[1:] = np.cumsum(chg)
    # istart[b, q]: chunk index (idx order) of block b's bucket-q run
    istart = np.zeros((NB, 4), dtype=np.int64)
    for g in range(NGRP):
        for q in range(4):
            pos = gstart[g] + kgq[g, :q].sum()
            for b in range(g * GRP, (g + 1) * GRP):
                istart[b, q] = pos
                pos += kbq[b, q]

    idx16 = np.zeros((NCORES, 128, NCH * 8), dtype=np.int16)
    dcol = np.full((NCORES, 128, NCH), 255, dtype=np.float32)
    for c in range(NCORES):
        m = core_of == c
        eb, eq, ed, es = blk_of[m], bucket_of[m], dcol_of[m], psrc[m]
        o = np.lexsort((es, eq, eb))
        eb, eq, ed, es = eb[o], eq[o], ed[o], es[o]
        pos = 0
        for b in range(NB):
            dch = dstart[b]
            for q in range(4):
                k = int(kbq[b, q])
                if k == 0:
                    continue
                cnt = int(counts[c, b, q])
                # row within quarter-table q
                esl = es[pos:pos + cnt]
                loc = esl - q * QSIZE
                flat_idx = np.zeros(k * 128, dtype=np.int16)
                flat_idx[:cnt] = loc.astype(np.int16)
                flat_dc = np.full(k * 128, 255, dtype=np.float32)
                flat_dc[:cnt] = ed[pos:pos + cnt].astype(np.float32)
                ich = istart[b, q]
                cols = flat_idx.reshape(k * 8, 16).T
                for gg in range(8):
                    idx16[c, gg * 16:(gg + 1) * 16,
                          ich * 8:(ich + k) * 8] = cols
                dcol[c, :, dch:dch + k] = flat_dc.reshape(k, 128).T
                pos += cnt
                dch += k
        assert pos == int(m.sum())

    dis_pad = np.zeros(PADN, dtype=np.float32)
    real = node_of_perm >= 0
    dis_pad[real] = dis[node_of_perm[real]]
    dis_cb = dis_pad.reshape(NCORES, NB, 128).transpose(0, 2, 1).copy()

    return dict(
        perm_of_node=perm_of_node, node_of_perm=node_of_perm, PADN=PADN,
        SHARD=SHARD, NB=NB, QSIZE=QSIZE, NCH=NCH,
        kbq=kbq, kgq=kgq, gstart=gstart, dstart=dstart, istart=istart,
        idx16=idx16, dcol=dcol, dis=dis, dis_cb=dis_cb,
    )


# ---------------- bass program builder ----------------


def build(params):
    NB = params["NB"]; NCH = params["NCH"]
    QSIZE = params["QSIZE"]; PADN = params["PADN"]
    KIN = params["KIN"]; F1 = params["F1"]; F2 = params["F2"]
    F3 = params["F3"]; COUT = params["COUT"]
    kbq = params["kbq"]; kgq = params["kgq"]
    gstart = params["gstart"]; dstart = params["dstart"]
    istart = params["istart"]
    REPEAT = params.get("repeat", 1)
    TIMING_LOOP = params.get("timing_loop", 0)
    MOCK_CC = params.get("mock_collectives", False)
    SHARD = NB * 128
    NCORES = 8
    KK = KIN // 128
    NGRP = NB // GRP
    WG = 14 if NB % 14 == 0 else (7 if NB % 7 == 0 else (2 if NB % 2 == 0 else 1))
    assert NB % WG == 0

    KB_MAX = int(kbq.sum(axis=1).max())

    nc = bacc.Bacc(num_swdge_queues=4)
    xT = nc.declare_dram_parameter("xT", [KIN, SHARD], BF, isOutput=False)
    w1 = nc.declare_dram_parameter("w1", [128, KK * TW], BF, isOutput=False)
    w2 = nc.declare_dram_parameter("w2", [F1, TW], BF, isOutput=False)
    w3 = nc.declare_dram_parameter("w3", [F2, TW], BF, isOutput=False)
    dcol_in = nc.declare_dram_parameter("dcol", [128, NCH], BF, isOutput=False)
    idx_in = nc.declare_dram_parameter("idx", [128, NCH * 8], I16, isOutput=False)
    iota_in = nc.declare_dram_parameter("iota", [128, 128 * KB_MAX], BF,
                                        isOutput=False)
    dis_in = nc.declare_dram_parameter("dis", [128, NB], FP, isOutput=False)
    dis2_in = nc.declare_dram_parameter("dis2", [128, NB], FP, isOutput=False)
    out_ext = nc.declare_dram_parameter("out", [SHARD, COUT], FP, isOutput=True)

    y_own = [nc.dram_tensor(f"y{l}_own", [SHARD, TW], BF) for l in (1, 2, 3)]
    y_full = [nc.dram_tensor(f"y{l}_full", [PADN, TW], BF,
                             addr_space="Shared") for l in (1, 2, 3)]
    rg = [list(range(NCORES))]

    CPC = params.get("cpc", 8)  # chunks per gather call (1024 idx max)
    SP = params.get("single_packet", True)
    MBUFS = params.get("mbufs", max(2, 64 // CPC))

    with TileContext(nc) as tc:
        with tc.tile_pool(name="const", bufs=1) as cpool, \
             tc.tile_pool(name="gt", bufs=1) as gtpool, \
             tc.tile_pool(name="ybuf", bufs=1) as ybpool, \
             tc.tile_pool(name="msg", bufs=MBUFS) as mpool, \
             tc.tile_pool(name="sgen", bufs=4) as spool, \
             tc.tile_pool(name="fin", bufs=3) as fpool, \
             tc.tile_pool(name="xs", bufs=2) as xspool, \
             tc.tile_pool(name="ps", bufs=2, space="PSUM") as pspool, \
             tc.tile_pool(name="pagg", bufs=4, space="PSUM") as papool, \
             tc.tile_pool(name="ptr", bufs=2, space="PSUM") as ptpool:

            ident = cpool.tile([128, 128], BF)
            make_identity(nc, ident[:])
            # iota_expT[p, d, j] = d  (materialized so S-gen's in1 has
            # unit stride on the last axis -> DVE 2x mode)
            iota = cpool.tile([128, 128, KB_MAX], BF)
            nc.sync.dma_start(
                out=iota[:], in_=iota_in[:].rearrange("p (d j) -> p d j",
                                                      j=KB_MAX))
            dcol = cpool.tile([128, NCH], BF)
            nc.sync.dma_start(out=dcol[:], in_=dcol_in[:])
            dis = cpool.tile([128, NB], FP)
            nc.sync.dma_start(out=dis[:], in_=dis_in[:])
            dis2 = cpool.tile([128, NB], FP)
            nc.sync.dma_start(out=dis2[:], in_=dis2_in[:])
            w1sb = cpool.tile([128, KK * TW], BF)
            nc.sync.dma_start(out=w1sb[:], in_=w1[:])
            w2sb = cpool.tile([F1, TW], BF)
            nc.sync.dma_start(out=w2sb[:], in_=w2[:])
            w3sb = cpool.tile([F2, TW], BF)
            nc.sync.dma_start(out=w3sb[:], in_=w3[:])
            # gather indices are layer-invariant: keep the whole table
            # resident instead of streaming per-group slices
            idxsb = cpool.tile([128, NCH * 8], I16)
            nc.sync.dma_start(out=idxsb[:], in_=idx_in[:])

            gT = gtpool.tile([128, SHARD], BF, tag="gT")
            ybuf = ybpool.tile([128, NB, TW], BF, tag="ybuf")

            SPLIT_AG = params.get("split_ag", 0)

            def allgather(l, part=None):
                if MOCK_CC:
                    for s in range(NCORES):
                        nc.sync.dma_start(
                            out=y_full[l][s * SHARD:(s + 1) * SHARD, :],
                            in_=y_own[l][:])
                elif SPLIT_AG:
                    H = SHARD // SPLIT_AG
                    parts = range(SPLIT_AG) if part is None else [part]
                    for h in parts:
                        nc.gpsimd.collective_compute(
                            "AllGather", mybir.AluOpType.bypass,
                            replica_groups=rg,
                            ins=[y_own[l][h * H:(h + 1) * H, :]],
                            outs=[y_full[l][:]
                                  .rearrange("(r s) f -> r s f", s=SHARD)
                                  [:, h * H:(h + 1) * H, :]])
                else:
                    nc.gpsimd.collective_compute(
                        "AllGather", mybir.AluOpType.bypass,
                        replica_groups=rg,
                        ins=[y_own[l][:]], outs=[y_full[l][:]])

            def y_write(l):
                for g in range(NB // WG):
                    nc.sync.dma_start(
                        out=y_own[l][g * WG * 128:(g + 1) * WG * 128, :]
                            .rearrange("(c p) f -> p c f", p=128),
                        in_=ybuf[:, g * WG:(g + 1) * WG, :])

            GATHER_ONLY = params.get("gather_only", False)
            NO_GATHER = params.get("no_gather", False)

            def agg_phase(l, F, last, outbuf=None):
                for g in range(NGRP):
                    # one ring tile per call; round-robin across the 4
                    # SWDGE queues (parallel desc-gen + ring drain).
                    msgs = [[] for _ in range(4)]
                    Ks = [int(kgq[g, q]) for q in range(4)]
                    offs = [int(gstart[g] + kgq[g, :q].sum())
                            for q in range(4)]
                    maxc = (max(Ks) + CPC - 1) // CPC
                    for ci in range(maxc):
                        for q in range(4):
                            c0 = ci * CPC
                            if c0 >= Ks[q]:
                                continue
                            n = min(CPC, Ks[q] - c0)
                            mt = mpool.tile([128, CPC, TW], BF, tag=f"msg{q}")
                            if NO_GATHER:
                                nc.vector.memset(mt[:, :n, :1], 0.0)
                            else:
                                nc.gpsimd.dma_gather(
                                    mt[:, :n, :],
                                    y_full[l][q * QSIZE:(q + 1) * QSIZE, :],
                                    idxsb[:, (offs[q] + c0) * 8:
                                          (offs[q] + c0 + n) * 8],
                                    n * 128, n * 128, TW, queue_num=q,
                                    single_packet=SP,
                                )
                            msgs[q].append(mt)
                    if GATHER_ONLY:
                        continue
                    for b in range(g * GRP, (g + 1) * GRP):
                        nkb = int(kbq[b].sum())
                        dch = int(dstart[b])
                        S = spool.tile([128, 128, KB_MAX], BF, tag="S")
                        if nkb:
                            nc.vector.tensor_tensor(
                                out=S[:, :, :nkb],
                                in0=dcol[:, dch:dch + nkb].unsqueeze(1)
                                    .to_broadcast([128, 128, nkb]),
                                in1=iota[:, :, :nkb],
                                op=mybir.AluOpType.is_equal,
                            )
                        pa = papool.tile([128, F], FP, tag="pa")
                        if nkb == 0:
                            nc.vector.memset(pa[:], 0.0)
                        done = 0
                        for q in range(4):
                            k = int(kbq[b, q])
                            moff = int(istart[b, q] - gstart[g] - kgq[g, :q].sum())
                            for j in range(k):
                                jj = moff + j
                                nc.tensor.matmul(
                                    pa[:], S[:, :, done],
                                    msgs[q][jj // CPC][:, jj % CPC, :F],
                                    start=(done == 0),
                                    stop=(done == nkb - 1))
                                done += 1
                        assert done == nkb
                        # self-loop: agg += own table rows (DVE, off the
                        # TensorE critical path)
                        pw = fpool.tile([128, F], FP, tag="pw")
                        nc.vector.tensor_tensor(
                            out=pw[:], in0=pa[:], in1=ybuf[:, b, :F],
                            op=mybir.AluOpType.add)
                        if not last:
                            gg = fpool.tile([128, F], BF, tag="g")
                            nc.scalar.activation(
                                gg[:], pw[:], mybir.ActivationFunctionType.Relu,
                                scale=dis2[:, b:b + 1])
                            pt = ptpool.tile([F, 128], BF, tag="pt")
                            nc.tensor.transpose(out=pt[:], in_=gg[:], identity=ident[:])
                            nc.vector.tensor_copy(gT[:F, b * 128:(b + 1) * 128], pt[:])
                        else:
                            z = fpool.tile([128, F], FP, tag="z")
                            nc.scalar.activation(
                                z[:], pw[:], mybir.ActivationFunctionType.Copy,
                                scale=dis[:, b:b + 1])
                            nm = fpool.tile([128, 1], FP, tag="nm")
                            nc.vector.tensor_reduce(
                                nm[:], z[:, :COUT], mybir.AxisListType.X,
                                mybir.AluOpType.max, negate=True)
                            e = fpool.tile([128, COUT], FP, tag="e")
                            nc.scalar.activation(
                                e[:], z[:, :COUT], mybir.ActivationFunctionType.Exp,
                                bias=nm[:])
                            s = fpool.tile([128, 1], FP, tag="s")
                            nc.vector.tensor_reduce(
                                s[:], e[:], mybir.AxisListType.X, mybir.AluOpType.add)
                            lg = fpool.tile([128, 1], FP, tag="lg")
                            nc.scalar.activation(
                                lg[:], s[:], mybir.ActivationFunctionType.Ln)
                            bb = fpool.tile([128, 1], FP, tag="bb")
                            nc.vector.tensor_tensor(
                                out=bb[:], in0=nm[:], in1=lg[:],
                                op=mybir.AluOpType.subtract)
                            nc.vector.tensor_scalar(
                                out=outbuf[:, b % WG, :], in0=z[:, :COUT],
                                scalar1=bb[:], scalar2=None, op0=mybir.AluOpType.add)
                            if b % WG == WG - 1:
                                g0 = b - (WG - 1)
                                nc.sync.dma_start(
                                    out=out_ext[g0 * 128:(b + 1) * 128, :]
                                        .rearrange("(c p) f -> p c f", p=128),
                                    in_=outbuf[:])
                                outbuf = fpool.tile([128, WG, COUT], FP, tag="ob")

            def pipeline(with_ag):
                if GATHER_ONLY:
                    agg_phase(0, F1, last=False)
                    agg_phase(1, F2, last=False)
                    agg_phase(2, F3, last=True)
                    return
                # ---- L1 y: stream bf16 x stripes per block-group ----
                WGY = 7
                for wg in range(NB // WGY):
                    c0 = wg * WGY * 128
                    stripes = []
                    for kk in range(KK):
                        st = xspool.tile([128, WGY * 128], BF, tag=f"xs{kk}")
                        nc.sync.dma_start(
                            out=st[:],
                            in_=xT[kk * 128:(kk + 1) * 128, c0:c0 + WGY * 128])
                        stripes.append(st)
                    for rl in range(WGY):
                        r = wg * WGY + rl
                        ps = pspool.tile([128, TW], FP, tag="psy")
                        for kk in range(KK):
                            nc.tensor.matmul(
                                ps[:], stripes[kk][:, rl * 128:(rl + 1) * 128],
                                w1sb[:, kk * TW:(kk + 1) * TW],
                                start=(kk == 0), stop=(kk == KK - 1))
                        nc.vector.tensor_copy(ybuf[:, r, :], ps[:])
                y_write(0)
                if with_ag: allgather(0)
                agg_phase(0, F1, last=False)

                # ---- L2 y ----
                for r in range(NB):
                    ps = pspool.tile([128, TW], FP, tag="psy")
                    nc.tensor.matmul(
                        ps[:], gT[:F1, r * 128:(r + 1) * 128], w2sb[:],
                        start=True, stop=True)
                    nc.vector.tensor_copy(ybuf[:, r, :], ps[:])
                y_write(1)
                if with_ag: allgather(1)
                agg_phase(1, F2, last=False)

                # ---- L3 y ----
                for r in range(NB):
                    ps = pspool.tile([128, TW], FP, tag="psy")
                    nc.tensor.matmul(
                        ps[:], gT[:F2, r * 128:(r + 1) * 128], w3sb[:],
                        start=True, stop=True)
                    nc.vector.tensor_copy(ybuf[:, r, :], ps[:])
                y_write(2)
                if with_ag: allgather(2)
                ob = fpool.tile([128, WG, COUT], FP, tag="ob")
                agg_phase(2, F3, last=True, outbuf=ob)

            for _rep in range(REPEAT):
                pipeline(not params.get("no_ag", False))
            if TIMING_LOOP:
                with tc.For_i(0, TIMING_LOOP, 1) as _:
                    pipeline(False)

    nc.compile()
    return nc


def make_inputs(x, W1, W2, W3, pp, KIN):
    import ml_dtypes
    N = x.shape[0]
    F1, F2 = W1.shape[1], W2.shape[1]
    COUT = W3.shape[1]
    SHARD, PADN, NB = pp["SHARD"], pp["PADN"], pp["NB"]
    perm = pp["perm_of_node"]
    KK = KIN // 128
    NCORES = 8
    bf = ml_dtypes.bfloat16

    xs = x.astype(np.float32) * pp["dis"][:, None]
    xs_p = np.zeros((PADN, KIN), np.float32)
    xs_p[perm] = xs
    KBM = int(pp["kbq"].sum(axis=1).max())
    iota = np.tile(np.repeat(np.arange(128, dtype=np.float32), KBM), (128, 1))
    w1p = np.zeros((KIN, TW), np.float32); w1p[:, :F1] = W1
    w1r = w1p.reshape(KK, 128, TW).transpose(1, 0, 2).reshape(128, KK * TW)
    w2p = np.zeros((F1, TW), np.float32); w2p[:, :F2] = W2
    w3p = np.zeros((F2, TW), np.float32); w3p[:, :COUT] = W3
    in_maps = []
    for c in range(NCORES):
        xT = np.ascontiguousarray(
            xs_p[c * SHARD:(c + 1) * SHARD, :].T).astype(bf)
        in_maps.append({
            "xT": xT,
            "w1": np.ascontiguousarray(w1r).astype(bf),
            "w2": np.ascontiguousarray(w2p).astype(bf),
            "w3": np.ascontiguousarray(w3p).astype(bf),
            "dcol": np.ascontiguousarray(pp["dcol"][c]).astype(bf),
            "idx": np.ascontiguousarray(pp["idx16"][c]),
            "iota": iota.astype(bf),
            "dis": np.ascontiguousarray(pp["dis_cb"][c]),
            "dis2": np.ascontiguousarray(pp["dis_cb"][c] ** 2),
        })
    return in_maps


_CACHE = {}


def _reference_numpy(x, edge_index, W1, b1, W2, b2, W3, b3):
    src = edge_index[0].astype(np.int64); dst = edge_index[1].astype(np.int64)
    N = x.shape[0]
    deg = np.bincount(dst, minlength=N) + 1.0
    dis = 1.0 / np.sqrt(deg)
    norm = (dis[src] * dis[dst]).astype(np.float32)

    def layer(xv, W, b):
        xw = xv @ W
        agg = np.zeros_like(xw)
        np.add.at(agg, dst, xw[src] * norm[:, None])
        agg += xw * (dis * dis)[:, None].astype(np.float32)
        return agg + b

    h1 = np.maximum(layer(x.astype(np.float32), W1, b1), 0)
    h2 = np.maximum(layer(h1, W2, b2), 0)
    z = layer(h2, W3, b3)
    m = z.max(axis=1, keepdims=True)
    return (z - m - np.log(np.exp(z - m).sum(axis=1, keepdims=True))).astype(np.float32)


def kernel(x, edge_index, W1, b1, W2, b2, W3, b3):
    x = np.asarray(x); edge_index = np.asarray(edge_index)
    W1 = np.asarray(W1, np.float32); W2 = np.asarray(W2, np.float32)
    W3 = np.asarray(W3, np.float32)
    b1 = np.asarray(b1, np.float32); b2 = np.asarray(b2, np.float32)
    b3 = np.asarray(b3, np.float32)
    if np.any(b1) or np.any(b2) or np.any(b3):
        # device kernel fuses the (spec-guaranteed zero) biases away
        return _reference_numpy(x, edge_index, W1, b1, W2, b2, W3, b3)

    KIN = x.shape[1]
    F1, F2 = W1.shape[1], W2.shape[1]
    COUT = W3.shape[1]
    pp = preprocess(edge_index, x.shape[0], NB_BLOCKS)
    in_maps = make_inputs(x, W1, W2, W3, pp, KIN)
    key = ("nc", pp["NCH"], pp["kbq"].tobytes())
    if key not in _CACHE:
        params = dict(NB=NB_BLOCKS, NCH=pp["NCH"], kbq=pp["kbq"],
                      kgq=pp["kgq"], gstart=pp["gstart"],
                      dstart=pp["dstart"], istart=pp["istart"],
                      QSIZE=pp["QSIZE"], PADN=pp["PADN"], KIN=KIN,
                      F1=F1, F2=F2, F3=64 if COUT <= 64 else 128, COUT=COUT)
        _CACHE[key] = build(params)
    nc = _CACHE[key]
    res = run_bass_kernel_spmd(nc, in_maps, list(range(NCORES)))
    full = np.concatenate([res.results[c]["out"] for c in range(NCORES)], axis=0)
    return np.ascontiguousarray(full[pp["perm_of_node"]]).astype(np.float32)
